# revision 38
# baseline (speedup 1.0000x reference)
"""Trainium2 Bass kernel for nn_CiBabyMambaHar (CI bidirectional Mamba HAR).

Self-contained: host-side weight prep (numpy) + Bass/Tile kernel builder +
SPMD runner over 8 NeuronCores (pure data parallel over batch).

v2: token-major selective scan with (d-group x state) partition passes,
packed weight blob, SBUF-resident LN stats, activation-table grouping.
"""
import numpy as np
import ml_dtypes

B, C, T = 256, 9, 128
D, S, NL, EXP, DTR, DCONV = 24, 16, 4, 2, 2, 4
DI = EXP * D  # 48
STEMK, PK, PS = 5, 16, 4
L = (T - PK) // PS + 1  # 29
NCLS = 6
EPS = 1e-5

NCORES = 8
NB = B // NCORES          # 32 batch rows per core
N = NB * C                # 288 sequences per core
NTOK = N * L              # 8352 tokens
CH = 16                   # seqs per matmul chunk
NCH = N // CH             # 18 chunks
CW = CH * L               # 464 chunk width
SC = 3 * CW               # 1392 super-chunk width (48 seqs)
NSC = NTOK // SC          # 6 super chunks
SCH = 3 * CH              # 48 seqs per super chunk
PADW = L + 6              # 35  (3 zero pad each side for conv windows)
TP = 96                   # partition count for LN stats relayout
TG = NTOK // TP           # 87
SGRP = 48                 # seqs per stem group
NSG = N // SGRP           # 6
NG = 6                    # scan d-groups (8 d each)
GD = DI // NG             # 8 d per group
HSW = 136                 # stem buffer width per seq


# ---------------- weight packing ----------------
# Each weight is placed in one of two blobs ([128, CB] bf16 / [128, CF] f32)
# at a (row=0, col=off) slot; the kernel slices views out of the blob tiles.

def _mk_layout():
    bf_items, f32_items = [], []

    def add(items, name, rows, cols):
        items.append((name, rows, cols))

    for cp in range(4):
        add(bf_items, f"stemW{cp}", 8, D)
    for m in range(4):
        add(bf_items, f"patchW4_{m}", 128, D)
    add(bf_items, "patch_bias", D, L)
    add(bf_items, "ones2a", D, 2)
    add(bf_items, "ones2b", D, 2)
    add(bf_items, "lnA_I", 3, D)
    add(bf_items, "lnB_I", 3, D)
    add(bf_items, "attnW", D, D)
    add(bf_items, "ctx", D, 1)
    add(bf_items, "headW", D, NCLS)
    add(bf_items, "onesD", 1, D)
    add(bf_items, "RepS", S, 128)
    for gi in range(NG):
        add(bf_items, f"RepNegS_{gi}", DI, 128)
        add(bf_items, f"RepD_{gi}", DI, 128)
        add(bf_items, f"SumS_{gi}", 128, DI)
    for i in range(NL):
        add(bf_items, f"WconvF_{i}", 56, 2 * DI)
        add(bf_items, f"WconvB_{i}", 56, 2 * DI)
        add(bf_items, f"Wz_{i}", D, DI)
        add(bf_items, f"WaugDT_{i}", DI, DI)
        add(bf_items, f"WBrep_{i}", DI, 128)
        add(bf_items, f"WCrep_{i}", DI, 128)
        add(bf_items, f"Wouty_{i}", DI, D)
        add(bf_items, f"Woutuc_{i}", DI, D)
        add(bf_items, f"lnA1_{i}", 3, D)
        add(bf_items, f"lnB1_{i}", 3, D)
        add(bf_items, f"lnA2_{i}", 3, D)
        add(bf_items, f"lnB2_{i}", 3, D)
    for i in range(NL - 1):
        add(bf_items, f"S6a_{i}", D, 6)
        add(bf_items, f"S6b_{i}", D, 6)
        add(bf_items, f"lnA2f_{i}", 7, D)
        add(bf_items, f"lnB2f_{i}", 7, D)
        add(bf_items, f"lnA1f_{i}", 7, D)
        add(bf_items, f"lnB1f_{i}", 7, D)

    add(f32_items, "stem_b4", 128, 1)
    add(f32_items, "head_bias", NCLS, 1)
    add(f32_items, "attn_b", D, 1)
    for i in range(NL):
        add(f32_items, f"conv_b_{i}", DI, 1)
        add(f32_items, f"dt_b_{i}", DI, 1)
    for i in range(NL - 1):
        add(f32_items, f"lnG_{i}", TP, 5)

    def assign(items):
        off = 0
        slots = {}
        for name, rows, cols in items:
            slots[name] = (rows, off, cols)
            off += cols
        return slots, off

    bf_slots, bf_w = assign(bf_items)
    f32_slots, f32_w = assign(f32_items)
    return bf_slots, bf_w, f32_slots, f32_w


BF_SLOTS, BF_W, F32_SLOTS, F32_W = _mk_layout()


def prep_weights(w):
    g = lambda k: np.asarray(w[k], np.float32)
    bf = ml_dtypes.bfloat16
    vals = {}

    bs = g("stem_bn_g") / np.sqrt(g("stem_bn_v") + EPS)
    stem_w = g("stem_w")[:, 0, :] * bs[:, None]    # [D, 5]
    # phase lhsT: out c = 4*cb+cp; X8 row j holds x[4*cb+j-2];
    # need sum_k w[k,d] x[4cb+cp+k-2] -> j = cp+k-2 => w row k = j-cp+2
    for cp in range(4):
        Wp = np.zeros((8, D), np.float32)
        for j in range(-2, 6):
            k = j - cp + 2
            if 0 <= k < STEMK:
                Wp[j + 2] = stem_w[:, k]
        vals[f"stemW{cp}"] = Wp
    sb = (g("stem_bn_b") - g("stem_bn_m") * bs)
    sb4 = np.zeros((128, 1), np.float32)
    for ph in range(4):
        sb4[32 * ph:32 * ph + D, 0] = sb
    vals["stem_b4"] = sb4
    pbs = g("patch_bn_g") / np.sqrt(g("patch_bn_v") + EPS)
    W_pp = g("pp_w")[:, :, 0] * pbs[:, None]
    b_patch = g("patch_bn_b") - g("patch_bn_m") * pbs
    pd = g("pd_w")[:, 0, :]
    patchW = np.stack([(W_pp * pd[None, :, j]).T for j in range(PK)], 0)
    # phase-major stacking: tap j=(4m+ph) contracts hsb96 rows 32ph..32ph+D
    for m in range(4):
        W4 = np.zeros((128, D), np.float32)
        for ph in range(4):
            W4[32 * ph:32 * ph + D, :] = patchW[4 * m + ph]
        vals[f"patchW4_{m}"] = W4
    vals["patch_bias"] = b_patch[:, None] + g("pos_embed")[0].T        # [24,29]
    A_chk = np.exp(g("A_log"))
    assert np.allclose(A_chk, np.broadcast_to(
        np.arange(1, S + 1, dtype=np.float32), (NL, DI, S)), atol=1e-3)

    one = np.ones(D, np.float32)
    zero = np.zeros(D, np.float32)
    vals["ones2a"] = np.stack([one, zero], 1)                          # [24,2]
    vals["ones2b"] = np.stack([zero, one], 1)
    # head LN: repA = rho, repB = mu*rho (no affine fold; head_g folded in headW)
    vals["lnA_I"] = np.stack([one, zero, zero], 0)
    vals["lnB_I"] = np.stack([zero, one, zero], 0)
    vals["attnW"] = g("attn_w").T
    vals["attn_b"] = g("attn_b")[:, None]
    vals["ctx"] = g("ctx")[:, None]
    hg = g("head_g"); hb = g("head_b")
    vals["headW"] = (g("head_w") * hg[None, :]).T                      # [24,6]
    vals["head_bias"] = (g("head_bias") + g("head_w") @ hb)[:, None]
    vals["onesD"] = np.ones((1, D), np.float32)

    # scan replication patterns; partition p = d_local*S + s_idx
    p = np.arange(128)
    dloc = p // S      # 0..7
    sidx = p % S       # 0..15
    RepS = np.zeros((S, 128), np.float32)
    for k in range(S):
        RepS[k, sidx == k] = 1.0
    vals["RepS"] = RepS
    for gi in range(NG):
        RepNegS = np.zeros((DI, 128), np.float32)
        RepD = np.zeros((DI, 128), np.float32)
        SumS = np.zeros((128, DI), np.float32)
        RepNegS[GD * gi + dloc, p] = -(sidx + 1).astype(np.float32)
        RepD[GD * gi + dloc, p] = 1.0
        SumS[p, GD * gi + dloc] = 1.0
        vals[f"RepNegS_{gi}"] = RepNegS
        vals[f"RepD_{gi}"] = RepD
        vals[f"SumS_{gi}"] = SumS

    sidx = np.arange(128) % S
    for i in range(NL):
        in_w = g("in_w")[i]
        W_u, W_z = in_w[:DI], in_w[DI:]
        cw = g("conv_w")[i][:, 0, :]
        g1 = g("ln1_g")[i]; b1 = g("ln1_b")[i]
        assert np.all(np.abs(g1) > 1e-6)
        c1 = b1 / g1
        Wc = np.stack([(W_u * cw[:, k:k + 1]).T * g1[:, None] for k in range(DCONV)], 0)
        # paired-tap conv: rhs rows 0:24 = hnpad, rows 32:56 = hnpad shifted
        # +1 col. fwd rhs offs (0,2): taps (0,1) and (2,3);
        # bwd rhs offs (5,3): taps (1,0) and (3,2).
        WF = np.zeros((56, 2 * DI), np.float32)
        WB56 = np.zeros((56, 2 * DI), np.float32)
        WF[0:D, 0:DI] = Wc[0]; WF[32:56, 0:DI] = Wc[1]
        WF[0:D, DI:] = Wc[2]; WF[32:56, DI:] = Wc[3]
        WB56[0:D, 0:DI] = Wc[1]; WB56[32:56, 0:DI] = Wc[0]
        WB56[0:D, DI:] = Wc[3]; WB56[32:56, DI:] = Wc[2]
        vals[f"WconvF_{i}"] = WF
        vals[f"WconvB_{i}"] = WB56
        vals[f"conv_b_{i}"] = g("conv_b")[i][:, None]
        vals[f"Wz_{i}"] = g("in_w")[i][DI:].T * g1[:, None]
        xp = g("xproj_w")[i]
        W_dtc = xp[:DTR, :].T @ g("dt_w")[i].T
        vals[f"WaugDT_{i}"] = W_dtc                                     # [48,48]
        BT = xp[DTR:DTR + S].T          # [48,16]
        CT = xp[DTR + S:].T             # [48,16]
        vals[f"WBrep_{i}"] = BT[:, sidx]                                # [48,128]
        vals[f"WCrep_{i}"] = CT[:, sidx]
        vals[f"dt_b_{i}"] = g("dt_b")[i][:, None]
        Dp = g("Dp")[i]; out_w = g("out_w")[i]
        vals[f"Wouty_{i}"] = out_w.T                                   # [48,24]
        vals[f"Woutuc_{i}"] = (out_w * Dp[None, :]).T
        # ln1 apply: hn = h16*rho - (mu*rho - c1)  (A=[1,0,0], B=[0,1,-c1])
        vals[f"lnA1_{i}"] = np.stack([one, zero, zero], 0)
        vals[f"lnB1_{i}"] = np.stack([zero, one, -c1], 0)
        g2 = g("ln2_g")[i]; b2 = g("ln2_b")[i]
        vals[f"lnA2_{i}"] = np.stack([g2, zero, zero], 0)
        vals[f"lnB2_{i}"] = np.stack([zero, g2, -b2], 0)

    # fused LN2(i)+LN1(i+1) weights
    for i in range(NL - 1):
        g2 = g("ln2_g")[i]; b2 = g("ln2_b")[i]
        g1n = g("ln1_g")[i + 1]; b1n = g("ln1_b")[i + 1]
        c1n = b1n / g1n
        # stats sums: from x: [1, g2, g2^2, g2*b2, 0, 0]; from x^2: [0,0,0,0, 1, g2^2]
        SA = np.stack([one, g2, g2 * g2, g2 * b2, zero, zero], 1)        # [D,6]
        SBq = np.stack([zero, zero, zero, zero, one, g2 * g2], 1)
        vals[f"S6a_{i}"] = SA
        vals[f"S6b_{i}"] = SBq
        consts = np.array([g2.sum(), (g2 * g2).sum(), (g2 * b2).sum(),
                           b2.sum(), (b2 * b2).sum()], np.float32)
        vals[f"lnG_{i}"] = np.broadcast_to(consts, (TP, 5)).copy()
        # st3 rows: 0=rho,1=mu*rho,2=1,3=rho*rho2,4=mu*rho*rho2,5=rho2,6=mu2*rho2
        z7 = np.zeros(D, np.float32)
        A2f = np.zeros((7, D), np.float32); A2f[0] = g2
        B2f = np.zeros((7, D), np.float32); B2f[1] = g2; B2f[2] = -b2
        A1f = np.zeros((7, D), np.float32); A1f[3] = g2
        B1f = np.zeros((7, D), np.float32)
        B1f[4] = g2; B1f[5] = -b2; B1f[6] = one; B1f[2] = -c1n
        vals[f"lnA2f_{i}"] = A2f
        vals[f"lnB2f_{i}"] = B2f
        vals[f"lnA1f_{i}"] = A1f
        vals[f"lnB1f_{i}"] = B1f

    blob_bf = np.zeros((128, BF_W), bf)
    for name, (rows, off, cols) in BF_SLOTS.items():
        v = vals[name]
        assert v.shape == (rows, cols), (name, v.shape, rows, cols)
        blob_bf[0:rows, off:off + cols] = v.astype(bf)
    blob_f32 = np.zeros((128, F32_W), np.float32)
    for name, (rows, off, cols) in F32_SLOTS.items():
        v = vals[name]
        assert v.shape == (rows, cols), (name, v.shape, rows, cols)
        blob_f32[0:rows, off:off + cols] = v.astype(np.float32)
    return {"wbf": blob_bf, "wf32": blob_f32}


def build(dbg=None):
    import os as _os
    STAGE = _os.environ.get("K_STAGE", "full")
    import concourse.bacc as bacc
    import concourse.tile as tile
    import concourse.bass as bass
    from concourse import mybir
    import contextlib

    F32, BF16 = mybir.dt.float32, mybir.dt.bfloat16
    A = mybir.AluOpType
    AF = mybir.ActivationFunctionType
    AX = mybir.AxisListType

    nc = bacc.Bacc("TRN2", target_bir_lowering=False, debug=False,
                   num_devices=NCORES)
    xin = nc.dram_tensor("x", [NB, T, C], BF16, kind="ExternalInput")
    yout = nc.dram_tensor("y", [NCLS, NB], F32, kind="ExternalOutput")
    wbf_d = nc.dram_tensor("wbf", [128, BF_W], BF16, kind="ExternalInput")
    wf32_d = nc.dram_tensor("wf32", [128, F32_W], F32, kind="ExternalInput")
    dbg = dbg or {}
    dbg_t = {name: nc.dram_tensor(name, shp, BF16 if d == "bf" else F32,
                                  kind="ExternalOutput")
             for name, (shp, d) in dbg.items()}

    with tile.TileContext(nc) as tc:
        ctx = contextlib.ExitStack()
        with ctx:
            W = ctx.enter_context(tc.tile_pool(name="wts", bufs=1))
            per = ctx.enter_context(tc.tile_pool(name="per", bufs=1))
            dram = ctx.enter_context(tc.tile_pool(name="dram", bufs=1, space="DRAM"))
            dst2 = dram.tile([6, NTOK], mybir.dt.bfloat16, tag="dst2")
            dst3 = dram.tile([6, NTOK], mybir.dt.bfloat16, tag="dst3")

            wbf = W.tile([128, BF_W], BF16, tag="wbf")
            wf32 = W.tile([128, F32_W], F32, tag="wf32")
            nc.sync.dma_start(wbf[:], wbf_d[:])
            nc.sync.dma_start(wf32[:], wf32_d[:])

            def wt(name):
                if name in BF_SLOTS:
                    rows, off, cols = BF_SLOTS[name]
                    return wbf[0:rows, off:off + cols]
                rows, off, cols = F32_SLOTS[name]
                return wf32[0:rows, off:off + cols]

            epsT = per.tile([128, 1], F32, tag="epsT")
            nc.vector.memset(epsT[:], EPS)
            oneT = per.tile([128, 1], F32, tag="oneT")
            nc.vector.memset(oneT[:], 1.0)

            hres = per.tile([D, N, L], F32, tag="hres")
            hres_f = hres[:].rearrange("d n p -> d (n p)")

            def dbg_dump(name, src_ap):
                if name in dbg_t:
                    nc.sync.dma_start(dbg_t[name][:], src_ap)

            # ================= stem + patch =================
            # X8[j, cb, n] = x[4*cb + j - 2, n]; hsb2[d, cp, cb, n] = stem
            # conv output at c = 4*cb + cp (post silu). Patch conv tap
            # (m, ph) reads hsb2[:, ph, m:m+L, nslice] -> psq free (l, n).
            with tc.tile_pool(name="stem1", bufs=1) as stp1, \
                 tc.tile_pool(name="stemps", bufs=4, space="PSUM") as stps, \
                 tc.tile_pool(name="stemdr", bufs=1, space="DRAM") as sdr:
                xt16 = stp1.tile([T, N], BF16, tag="xt16")
                nc.sync.dma_start(
                    xt16[:].rearrange("t (b c) -> t b c", b=NB),
                    bass.AP(tensor=xin, offset=0,
                            ap=[[C, T], [T * C, NB], [1, C]]))
                zrow = stp1.tile([2, N], BF16, tag="zrow")
                nc.vector.memset(zrow[:], 0.0)
                xdr = sdr.tile([132, N], BF16, tag="xdr")
                nc.sync.dma_start(xdr[2:130, :], xt16[:])
                nc.sync.dma_start(xdr[0:2, :], zrow[:])
                nc.sync.dma_start(xdr[130:132, :], zrow[:])
                X8 = stp1.tile([8, 32, N], BF16, tag="X8")
                nc.sync.dma_start(
                    X8[:], bass.AP(tensor=xdr.tensor, offset=0,
                                   ap=[[N, 8], [4 * N, 32], [1, N]]))
                hsb96 = stp1.tile([128, 33, N], BF16, tag="hsb96")
                nc.vector.memset(hsb96[:], 0.0)
                for cp in range(4):
                    for ns in range(NCH):
                        n0 = CH * ns
                        pst = stps.tile([128, 32 * CH], F32, tag="pss")
                        nc.tensor.matmul(
                            pst[0:D, 0:32 * CH],
                            wt(f"stemW{cp}"),
                            X8[:, :, n0:n0 + CH],
                            start=True, stop=True)
                        nc.scalar.activation(
                            hsb96[32 * cp:32 * cp + D, 1:33, n0:n0 + CH],
                            pst[0:D, 0:32 * CH].rearrange(
                                "d (cb n) -> d cb n", n=CH),
                            AF.Silu, bias=wt("stem_b4")[32 * cp:32 * cp + D, :],
                            scale=1.0)
                for c in range(NCH):
                    n0 = CH * c
                    psq = stps.tile([128, 32 * CH], F32, tag="pss")
                    for m in range(4):
                        nc.tensor.matmul(
                            psq[0:D, 0:CW],
                            wt(f"patchW4_{m}"),
                            hsb96[:, m:m + L, n0:n0 + CH],
                            start=(m == 0), stop=(m == 3))
                    nc.vector.tensor_tensor(
                        hres[:, n0:n0 + CH, :].transpose([0, 2, 1]),
                        psq[0:D, 0:CW].rearrange("d (p n) -> d p n", n=CH),
                        wt("patch_bias").unsqueeze(2).broadcast_to([D, L, CH]),
                        A.add)
            dbg_dump("d_h0", hres_f)

            psp = ctx.enter_context(tc.tile_pool(name="psum", bufs=5, space="PSUM"))
            psy = ctx.enter_context(tc.tile_pool(name="psumy", bufs=2, space="PSUM"))
            rng = ctx.enter_context(tc.tile_pool(name="rng", bufs=3))
            sm1 = ctx.enter_context(tc.tile_pool(name="small1", bufs=1))
            per2 = ctx.enter_context(tc.tile_pool(name="per2", bufs=1))

            hnpad = per2.tile([56, N, PADW], BF16, tag="hnpad")
            nc.vector.memset(hnpad[:], 0.0)
            ucF = per2.tile([DI, NTOK], BF16, tag="ucF")
            ucB = per2.tile([DI, NTOK], BF16, tag="ucB")
            szt = per2.tile([DI, NTOK], BF16, tag="szt")
            dtT = per2.tile([DI, NTOK], BF16, tag="dtT")
            dtTb = per2.tile([DI, NTOK], BF16, tag="dtTb")
            # h16 scratch aliases dtTb rows 0:D (disjoint lifetimes)
            stc_all = dtT  # stats scratch: disjoint lifetime with scan's dtT use

            st3_all = per2.tile([7, NTOK], BF16, tag="st3_all")
            nc.vector.memset(st3_all[:], 1.0)
            hw16 = ucF  # dead after layers; reuse rows 0:D as scratch

            # ---------- LN helpers ----------
            def ln_stats_chunk(src_f32, sl, wa, wb, nrows, with_h16=True):
                """one CW sub-chunk of LN stats: h16/hsq + weighted-sum matmuls
                into stc_all rows [0:nrows]."""
                if with_h16:
                    nc.scalar.activation(dtTb[0:D, sl], src_f32[:, sl],
                                         AF.Copy, scale=1.0)
                hsq = rng.tile([D, CW], BF16, tag="hsq", bufs=2)
                nc.scalar.square(hsq[:], dtTb[0:D, sl])
                psS = psp.tile([128, CW], F32, tag="ps", name="psS")
                nc.tensor.matmul(psS[0:nrows, :], wa, dtTb[0:D, sl],
                                 start=True, stop=False)
                nc.tensor.matmul(psS[0:nrows, :], wb, hsq[:],
                                 start=False, stop=True)
                nc.vector.tensor_copy(stc_all[0:nrows, sl], psS[0:nrows, :])

            def ln_stats(src_f32, with_h16=True):
                """src [D, NTOK] f32 -> st3_all rows [rho, mu*rho] bf16 (SBUF);
                also refreshes h16 (bf16 copy of src) when with_h16."""
                for c in range(NCH):
                    sl = slice(CW * c, CW * (c + 1))
                    ln_stats_chunk(src_f32, sl, wt("ones2a"), wt("ones2b"), 2,
                                   with_h16)
                ln_stats_finish()

            def ln_stats_finish():
                nc.sync.dma_start(dst2[0:2, :], stc_all[0:2, :])
                tps = sm1.tile([TP, 2, TG], BF16, tag="tps")
                nc.sync.dma_start(
                    tps[:], bass.AP(tensor=dst2.tensor, offset=0,
                                    ap=[[TG, TP], [NTOK, 2], [1, TG]]))
                mu = sm1.tile([TP, TG], F32, tag="mu")
                var = sm1.tile([TP, TG], F32, tag="var")
                t1 = sm1.tile([TP, TG], F32, tag="t1")
                nc.scalar.mul(mu[:], tps[:, 0, :], 1.0 / D)
                nc.scalar.mul(var[:], tps[:, 1, :], 1.0 / D)
                nc.vector.tensor_tensor(t1[:], mu[:], mu[:], A.mult)
                nc.vector.tensor_tensor(var[:], var[:], t1[:], A.subtract)
                nc.scalar.activation(var[:], var[:], AF.Sqrt,
                                     bias=epsT[0:TP, :], scale=1.0)
                nc.vector.reciprocal(var[:], var[:])
                nc.vector.tensor_tensor(t1[:], mu[:], var[:], A.mult)
                st3 = sm1.tile([TP, 2, TG], BF16, tag="st3")
                nc.vector.tensor_copy(st3[:, 0, :], var[:])
                nc.vector.tensor_copy(st3[:, 1, :], t1[:])
                nc.sync.dma_start(
                    bass.AP(tensor=dst3.tensor, offset=0,
                            ap=[[TG, TP], [NTOK, 2], [1, TG]]), st3[:])
                nc.sync.dma_start(st3_all[0:2, :], dst3[0:2, :])

            def ln_apply(lhsA, lhsB, dst_kind, rows=3):
                """reconstruct repA/repB per chunk and apply:
                dst_kind 'hnpad': hnpad = h16*repA - repB  (bf16, padded view)
                dst_kind 'hres':  hres  = h16*repA - repB  (f32)"""
                for c in range(NCH):
                    sl = slice(CW * c, CW * (c + 1))
                    psR = psp.tile([128, CW], F32, tag="ps")
                    nc.tensor.matmul(psR[0:D, :], lhsA, st3_all[0:rows, sl],
                                     start=True, stop=True)
                    psR2 = psp.tile([128, CW], F32, tag="ps")
                    nc.tensor.matmul(psR2[0:D, :], lhsB, st3_all[0:rows, sl],
                                     start=True, stop=True)
                    t16 = rng.tile([D, CW], BF16, tag="t16", bufs=2)
                    nc.vector.tensor_tensor(t16[:], dtTb[0:D, sl], psR[0:D, :], A.mult)
                    if dst_kind == "hnpad":
                        nc.vector.tensor_tensor(
                            hnpad[0:D, CH * c:CH * (c + 1), 3:32],
                            t16[:].rearrange("d (n p) -> d n p", n=CH),
                            psR2[0:D, :].rearrange("d (n p) -> d n p", n=CH),
                            A.subtract)
                        nc.sync.dma_start(
                            hnpad[32:56, CH * c:CH * (c + 1), 2:32],
                            hnpad[0:D, CH * c:CH * (c + 1), 3:33])
                    else:
                        nc.vector.tensor_tensor(
                            hres[:, CH * c:CH * (c + 1), :],
                            t16[:].rearrange("d (n p) -> d n p", n=CH),
                            psR2[0:D, :].rearrange("d (n p) -> d n p", n=CH),
                            A.subtract)

            def ln_stats6_finish(li):
                """fused LN2(li)+LN1(li+1) stats finish (stats already in
                stc_all rows 0:6): transpose, rho/rho2, -> st3_all."""
                nc.sync.dma_start(dst2[:], stc_all[0:6, :])
                tps = sm1.tile([TP, 6, TG], BF16, tag="tps6")
                nc.sync.dma_start(
                    tps[:], bass.AP(tensor=dst2.tensor, offset=0,
                                    ap=[[TG, TP], [NTOK, 6], [1, TG]]))
                G = wt(f"lnG_{li}")

                def gbc(k):
                    return G[:, k:k + 1].broadcast_to([TP, TG])

                f32t = lambda tag: sm1.tile([TP, TG], F32, tag=tag, name=tag)
                mu = f32t("mu"); var = f32t("var"); t1 = f32t("t1")
                t2 = f32t("t2"); t3 = f32t("t3"); rho = f32t("rho")
                nc.scalar.mul(mu[:], tps[:, 0, :], 1.0 / D)
                nc.scalar.mul(var[:], tps[:, 4, :], 1.0 / D)
                nc.vector.tensor_tensor(t1[:], mu[:], mu[:], A.mult)
                nc.vector.tensor_tensor(var[:], var[:], t1[:], A.subtract)
                nc.scalar.activation(var[:], var[:], AF.Sqrt,
                                     bias=epsT[0:TP, :], scale=1.0)
                nc.vector.reciprocal(rho[:], var[:])
                # Sa = rho*(S1 - mu*G0) + B0;  mu2 = Sa/D
                nc.vector.tensor_tensor(t1[:], mu[:], gbc(0), A.mult)
                nc.vector.tensor_tensor(t1[:], tps[:, 1, :], t1[:], A.subtract)
                nc.vector.tensor_tensor(t1[:], t1[:], rho[:], A.mult)
                nc.vector.tensor_tensor(t1[:], t1[:], gbc(3), A.add)
                mu2 = f32t("mu2")
                nc.scalar.mul(mu2[:], t1[:], 1.0 / D)
                # Saa = rho^2*(S5 - 2mu*S2 + mu^2*G1) + 2rho*(S3 - mu*G2) + B1
                nc.vector.tensor_tensor(t1[:], mu[:], tps[:, 2, :], A.mult)
                nc.scalar.mul(t1[:], t1[:], 2.0)
                nc.vector.tensor_tensor(t1[:], tps[:, 5, :], t1[:], A.subtract)
                nc.vector.tensor_tensor(t2[:], mu[:], mu[:], A.mult)
                nc.vector.tensor_tensor(t2[:], t2[:], gbc(1), A.mult)
                nc.vector.tensor_tensor(t1[:], t1[:], t2[:], A.add)
                nc.vector.tensor_tensor(t2[:], rho[:], rho[:], A.mult)
                nc.vector.tensor_tensor(t1[:], t1[:], t2[:], A.mult)
                nc.vector.tensor_tensor(t2[:], mu[:], gbc(2), A.mult)
                nc.vector.tensor_tensor(t2[:], tps[:, 3, :], t2[:], A.subtract)
                nc.vector.tensor_tensor(t2[:], t2[:], rho[:], A.mult)
                nc.scalar.mul(t2[:], t2[:], 2.0)
                nc.vector.tensor_tensor(t1[:], t1[:], t2[:], A.add)
                nc.vector.tensor_tensor(t1[:], t1[:], gbc(4), A.add)
                nc.scalar.mul(t1[:], t1[:], 1.0 / D)
                nc.vector.tensor_tensor(t2[:], mu2[:], mu2[:], A.mult)
                nc.vector.tensor_tensor(t1[:], t1[:], t2[:], A.subtract)
                nc.scalar.activation(t1[:], t1[:], AF.Sqrt,
                                     bias=epsT[0:TP, :], scale=1.0)
                rho2 = f32t("rho2")
                nc.vector.reciprocal(rho2[:], t1[:])
                st6 = sm1.tile([TP, 6, TG], BF16, tag="st6")
                nc.vector.tensor_copy(st6[:, 0, :], rho[:])
                nc.vector.tensor_tensor(t2[:], mu[:], rho[:], A.mult)
                nc.vector.tensor_copy(st6[:, 1, :], t2[:])
                nc.vector.tensor_tensor(t3[:], rho[:], rho2[:], A.mult)
                nc.vector.tensor_copy(st6[:, 2, :], t3[:])
                nc.vector.tensor_tensor(t3[:], t2[:], rho2[:], A.mult)
                nc.vector.tensor_copy(st6[:, 3, :], t3[:])
                nc.vector.tensor_copy(st6[:, 4, :], rho2[:])
                nc.vector.tensor_tensor(t3[:], mu2[:], rho2[:], A.mult)
                nc.vector.tensor_copy(st6[:, 5, :], t3[:])
                nc.sync.dma_start(
                    bass.AP(tensor=dst3.tensor, offset=0,
                            ap=[[TG, TP], [NTOK, 6], [1, TG]]), st6[:])
                nc.sync.dma_start(st3_all[0:2, :], dst3[0:2, :])
                nc.sync.dma_start(st3_all[3:7, :], dst3[2:6, :])

            # ================= layers =================
            NL_eff = 0 if STAGE == "stem" else (1 if STAGE.startswith("l0") else NL)
            nc.vector.memset(hnpad[:, :, 0:3], 0.0)
            nc.vector.memset(hnpad[:, :, 32:35], 0.0)
            for li in range(NL_eff):
                # stats for this layer's tail LN are inlined into pass-2
                # (rev==1) per super-chunk; only the finish stays serial.
                if li < NL - 1:
                    li_stats = (wt(f"S6a_{li}"), wt(f"S6b_{li}"), 6)
                else:
                    li_stats = (wt("ones2a"), wt("ones2b"), 2)
                # ---- LN1 (layer 0 only; later layers get hnpad from the
                # fused LN at the previous layer's tail) ----
                if li == 0:
                    ln_stats(hres_f)
                    ln_apply(wt(f"lnA1_{li}"), wt(f"lnB1_{li}"), "hnpad")
                if li == 0:
                    dbg_dump("d_hn0", hnpad[0:D, :, 3:32])

                # ---- conv+uproj (f/b) + z ----
                if STAGE == "l0ln1":
                    break
                for c in range(NCH):
                    for rev in (0, 1):
                        psC = psp.tile([128, CW], F32, tag="ps")
                        Wcp = wt(f"WconvF_{li}") if not rev else wt(f"WconvB_{li}")
                        offs = (0, 2) if not rev else (5, 3)
                        for k2 in range(2):
                            nc.tensor.matmul(
                                psC[0:DI, :],
                                Wcp[:, k2 * DI:(k2 + 1) * DI],
                                hnpad[0:56, CH * c:CH * (c + 1), offs[k2]:offs[k2] + L],
                                start=(k2 == 0), stop=(k2 == 1))
                        nc.scalar.activation(
                            (ucF if not rev else ucB)[:, CW * c:CW * (c + 1)],
                            psC[0:DI, :], AF.Silu,
                            bias=wt(f"conv_b_{li}"), scale=1.0)
                    psZ = psp.tile([128, CW], F32, tag="ps")
                    nc.tensor.matmul(psZ[0:DI, :], wt(f"Wz_{li}"),
                                     hnpad[0:D, CH * c:CH * (c + 1), 3:32],
                                     start=True, stop=True)
                    nc.scalar.activation(szt[:, CW * c:CW * (c + 1)],
                                         psZ[0:DI, :], AF.Silu)
                if li == 0:
                    dbg_dump("d_uc0", ucF[:])

                # ---- per direction: xproj + scan (token-major S-layout) ----
                if STAGE == "l0conv":
                    break
                for rev in (0, 1):
                    uct = ucF if not rev else ucB
                    dtc = dtT if not rev else dtTb
                    # pass 1: dt = softplus(xproj); all-Exp sweep then
                    # all-Ln sweep so the act table is stable per sweep
                    for c in range(NCH):
                        sl = slice(CW * c, CW * (c + 1))
                        psD = psp.tile([128, CW], F32, tag="ps")
                        nc.tensor.matmul(psD[0:DI, :], wt(f"WaugDT_{li}"),
                                         uct[:, sl], start=True, stop=True)
                        nc.scalar.activation(dtc[:, sl], psD[0:DI, :], AF.Exp,
                                             bias=wt(f"dt_b_{li}"), scale=1.0)
                    for c2 in range(NSC):
                        s2 = slice(SC * c2, SC * (c2 + 1))
                        nc.scalar.activation(dtc[:, s2], dtc[:, s2], AF.Ln,
                                             bias=oneT[0:DI, :], scale=1.0)
                for rev in (0, 1):
                    uct = ucF if not rev else ucB
                    dtc = dtT if not rev else dtTb
                    # pass 2: B/C + scan, super-chunked at SC=1392 (48 seqs)
                    if STAGE == "l0p1":
                        continue
                    for c2 in range(NSC):
                        s2 = slice(SC * c2, SC * (c2 + 1))
                        subs = [slice(SC * c2 + CW * k, SC * c2 + CW * (k + 1))
                                for k in range(3)]
                        dtu16 = rng.tile([DI, SC], BF16, tag="dtu16", bufs=2)
                        nc.gpsimd.tensor_tensor(dtu16[:], dtc[:, s2],
                                                uct[:, s2], A.mult)
                        # B/C replicated to (d,s) partitions straight from uc
                        Brep = rng.tile([128, SC], BF16, tag="Brep", bufs=1)
                        Crep = rng.tile([128, SC], BF16, tag="Crep", bufs=1)
                        for k in range(3):
                            ck = slice(CW * k, CW * (k + 1))
                            psR = psp.tile([128, CW], F32, tag="ps")
                            nc.tensor.matmul(psR[:], wt(f"WBrep_{li}"),
                                             uct[:, subs[k]], start=True, stop=True)
                            nc.scalar.activation(Brep[:, ck], psR[:], AF.Copy,
                                                 scale=1.0)
                            psR2 = psp.tile([128, CW], F32, tag="ps")
                            nc.tensor.matmul(psR2[:], wt(f"WCrep_{li}"),
                                             uct[:, subs[k]], start=True, stop=True)
                            nc.scalar.activation(Crep[:, ck], psR2[:], AF.Copy,
                                                 scale=1.0)
                        psY = [psy.tile([DI, CW], F32, tag=f"psY{k}", bufs=1,
                                        name=f"psY{k}")
                               for k in range(3)]
                        for gi in range(NG):
                            dA16 = rng.tile([128, SC], BF16, tag="dA16", bufs=2)
                            dbu16 = rng.tile([128, SC], BF16, tag="dbu16", bufs=2)
                            psAs, psUs = [], []
                            for k in range(3):
                                psA = psp.tile([128, CW], F32, tag="ps",
                                               name="psA")
                                nc.tensor.matmul(psA[:], wt(f"RepNegS_{gi}"),
                                                 dtc[:, subs[k]], start=True,
                                                 stop=True)
                                psAs.append(psA)
                            for k in range(3):
                                psU = psp.tile([128, CW], F32, tag="ps",
                                               name="psU")
                                nc.tensor.matmul(psU[:], wt(f"RepD_{gi}"),
                                                 dtu16[:, CW * k:CW * (k + 1)],
                                                 start=True, stop=True)
                                psUs.append(psU)
                            # zero seq-boundary columns early (off the
                            # exp->scan path); Exp writes skip l=0 so no WAW
                            nc.vector.memset(
                                dA16[:].rearrange("p (n l) -> p n l", l=L)[:, :, 0:1],
                                0.0)
                            for k in range(3):
                                ck = slice(CW * k, CW * (k + 1))
                                nc.scalar.activation(
                                    dA16[:, ck].rearrange(
                                        "p (n l) -> p n l", l=L)[:, :, 1:L],
                                    psAs[k][:].rearrange(
                                        "p (n l) -> p n l", l=L)[:, :, 1:L],
                                    AF.Exp, scale=1.0)
                                nc.vector.tensor_tensor(dbu16[:, ck], psUs[k][:],
                                                        Brep[:, ck], A.mult)
                            sc16 = rng.tile([128, SC], BF16, tag="sc16", bufs=2)
                            nc.vector.tensor_tensor_scan(
                                sc16[:], dA16[:], dbu16[:], 0.0, A.mult, A.add)
                            # hc reuses the dead dA16 buffer
                            nc.gpsimd.tensor_tensor(dA16[:], sc16[:],
                                                    Crep[:], A.mult)
                            for k in range(3):
                                ck = slice(CW * k, CW * (k + 1))
                                nc.tensor.matmul(psY[k][:], wt(f"SumS_{gi}"),
                                                 dA16[:, ck], start=(gi == 0),
                                                 stop=(gi == NG - 1))
                        # gate with silu(z); bwd output is seq-reversed -> undo
                        for k in range(3):
                            sl = subs[k]
                            yg = rng.tile([DI, CW], BF16, tag="yg", bufs=2)
                            ucg = rng.tile([DI, CW], BF16, tag="ucg", bufs=2)
                            if not rev:
                                nc.vector.tensor_tensor(yg[:], psY[k][:],
                                                        szt[:, sl], A.mult)
                                nc.gpsimd.tensor_tensor(ucg[:], uct[:, sl],
                                                        szt[:, sl], A.mult)
                            else:
                                nc.vector.tensor_tensor(
                                    yg[:].rearrange("d (n l) -> d n l", n=CH),
                                    psY[k][:].rearrange("d (n l) -> d n l", n=CH)[:, :, ::-1],
                                    szt[:, sl].rearrange("d (n l) -> d n l", n=CH),
                                    A.mult)
                                nc.gpsimd.tensor_tensor(
                                    ucg[:].rearrange("d (n l) -> d n l", n=CH),
                                    uct[:, sl].rearrange("d (n l) -> d n l", n=CH)[:, :, ::-1],
                                    szt[:, sl].rearrange("d (n l) -> d n l", n=CH),
                                    A.mult)
                            psO = psp.tile([128, CW], F32, tag="ps")
                            nc.tensor.matmul(psO[0:D, :], wt(f"Wouty_{li}"),
                                             yg[:], start=True, stop=False)
                            nc.tensor.matmul(psO[0:D, :], wt(f"Woutuc_{li}"),
                                             ucg[:], start=False, stop=True)
                            nc.vector.tensor_tensor(hres_f[:, sl], hres_f[:, sl],
                                                    psO[0:D, :], A.add)
                        if rev and li_stats is not None:
                            wa, wb, nrows = li_stats
                            for k in range(3):
                                ln_stats_chunk(hres_f, subs[k], wa, wb, nrows)
                # ---- LN2 -> new hres (+ fused LN1 of next layer) ----
                if STAGE == "l0scan":
                    break
                if li < NL - 1:
                    ln_stats6_finish(li)
                    ln_apply(wt(f"lnA1f_{li}"), wt(f"lnB1f_{li}"), "hnpad",
                             rows=7)
                    ln_apply(wt(f"lnA2f_{li}"), wt(f"lnB2f_{li}"), "hres",
                             rows=7)
                else:
                    ln_stats_finish()
                    ln_apply(wt(f"lnA2_{li}"), wt(f"lnB2_{li}"), "hres")
                dbg_dump(f"d_hL{li}", hres_f)

            # ================= attention pool + head =================
            if STAGE.endswith("nohead"):
                nc.sync.dma_start(yout[:], hres[0:NCLS, 0:NB, 0])
            else:
                for c in range(NCH):
                    sl = slice(CW * c, CW * (c + 1))
                    nc.scalar.activation(dtTb[0:D, sl], hres_f[:, sl], AF.Copy, scale=1.0)
                    psA_ = psp.tile([128, CW], F32, tag="ps")
                    nc.tensor.matmul(psA_[0:D, :], wt("attnW"), dtTb[0:D, sl],
                                     start=True, stop=True)
                    uat = rng.tile([D, CW], BF16, tag="uat", bufs=2)
                    nc.scalar.activation(uat[:], psA_[0:D, :], AF.Tanh,
                                         bias=wt("attn_b"), scale=1.0)
                    psSc = psp.tile([128, CW], F32, tag="ps")
                    nc.tensor.matmul(psSc[0:1, :], wt("ctx"), uat[:],
                                     start=True, stop=True)
                    nc.vector.tensor_copy(stc_all[0:1, sl], psSc[0:1, :])
                nc.sync.dma_start(dst2[0:1, :], stc_all[0:1, :])
                ssc = sm1.tile([TP, 3, L], BF16, tag="ssc")
                nc.sync.dma_start(
                    ssc[:], bass.AP(tensor=dst2.tensor, offset=0,
                                    ap=[[3 * L, TP], [L, 3], [1, L]]))
                smax = sm1.tile([TP, 3], BF16, tag="smax")
                nc.vector.tensor_reduce(smax[:], ssc[:], AX.X, A.max)
                nc.vector.tensor_tensor(
                    ssc[:], ssc[:], smax[:].unsqueeze(2).broadcast_to([TP, 3, L]),
                    A.subtract)
                nc.scalar.activation(ssc[:], ssc[:], AF.Exp, scale=1.0)
                ssum = sm1.tile([TP, 3], F32, tag="ssum")
                nc.vector.tensor_reduce(ssum[:], ssc[:], AX.X, A.add)
                nc.vector.reciprocal(ssum[:], ssum[:])
                nc.vector.tensor_tensor(
                    ssc[:], ssc[:], ssum[:].unsqueeze(2).broadcast_to([TP, 3, L]),
                    A.mult)
                asc16 = sm1.tile([TP, 3, L], BF16, tag="asc16")
                nc.vector.tensor_copy(asc16[:], ssc[:])
                nc.sync.dma_start(
                    bass.AP(tensor=dst3.tensor, offset=0,
                            ap=[[3 * L, TP], [L, 3], [1, L]]), asc16[:])
                nc.sync.dma_start(st3_all[0:1, :], dst3[0:1, :])
                for c in range(NCH):
                    sl = slice(CW * c, CW * (c + 1))
                    psL = psp.tile([128, CW], F32, tag="ps")
                    nc.tensor.matmul(psL[0:D, :], wt("onesD"), st3_all[0:1, sl],
                                     start=True, stop=True)
                    nc.vector.tensor_tensor(hw16[0:D, sl], dtTb[0:D, sl], psL[0:D, :],
                                            A.mult)
                cpool = sm1.tile([D, N], F32, tag="cpool")
                nc.vector.tensor_reduce(
                    cpool[:], hw16[0:D, :].rearrange("d (n p) -> d n p", n=N), AX.X, A.add)
                cmean = sm1.tile([D, NB], F32, tag="cmean")
                nc.vector.tensor_reduce(
                    cmean[:], cpool[:].rearrange("d (b c) -> d b c", b=NB), AX.X, A.add)
                nc.scalar.mul(cmean[:], cmean[:], 1.0 / C)
                c16 = sm1.tile([D, NB], BF16, tag="c16")
                csq16 = sm1.tile([D, NB], BF16, tag="csq16")
                nc.scalar.activation(c16[:], cmean[:], AF.Copy, scale=1.0)
                nc.scalar.square(csq16[:], cmean[:])
                psSh = psp.tile([128, CW], F32, tag="ps")
                nc.tensor.matmul(psSh[0:2, 0:NB], wt("ones2a"), c16[:],
                                 start=True, stop=False)
                nc.tensor.matmul(psSh[0:2, 0:NB], wt("ones2b"), csq16[:],
                                 start=False, stop=True)
                sAB = sm1.tile([2, NB], F32, tag="sAB")
                nc.vector.tensor_copy(sAB[:], psSh[0:2, 0:NB])
                sB0 = sm1.tile([1, NB], F32, tag="sB0")
                nc.sync.dma_start(sB0[:], sAB[1:2, :])
                hmu = sm1.tile([1, NB], F32, tag="hmu")
                hvar = sm1.tile([1, NB], F32, tag="hvar")
                hm2 = sm1.tile([1, NB], F32, tag="hm2")
                nc.scalar.mul(hmu[:], sAB[0:1, :], 1.0 / D)
                nc.scalar.mul(hvar[:], sB0[:], 1.0 / D)
                nc.vector.tensor_tensor(hm2[:], hmu[:], hmu[:], A.mult)
                nc.vector.tensor_tensor(hvar[:], hvar[:], hm2[:], A.subtract)
                nc.scalar.activation(hvar[:], hvar[:], AF.Sqrt,
                                     bias=epsT[0:1, :], scale=1.0)
                nc.vector.reciprocal(hvar[:], hvar[:])
                nc.vector.tensor_tensor(hm2[:], hmu[:], hvar[:], A.mult)
                r3 = sm1.tile([3, NB], BF16, tag="r3")
                r1b = sm1.tile([1, NB], BF16, tag="r1b")
                nc.vector.tensor_copy(r1b[:], hvar[:])
                nc.sync.dma_start(r3[0:1, :], r1b[:])
                nc.vector.tensor_copy(r1b[:], hm2[:])
                nc.sync.dma_start(r3[1:2, :], r1b[:])
                nc.vector.memset(r1b[:], 1.0)
                nc.sync.dma_start(r3[2:3, :], r1b[:])
                psRA = psp.tile([128, CW], F32, tag="ps")
                nc.tensor.matmul(psRA[0:D, 0:NB], wt("lnA_I"), r3[:],
                                 start=True, stop=True)
                psRB = psp.tile([128, CW], F32, tag="ps")
                nc.tensor.matmul(psRB[0:D, 0:NB], wt("lnB_I"), r3[:],
                                 start=True, stop=True)
                cn1 = sm1.tile([D, NB], F32, tag="cn1")
                nc.vector.tensor_tensor(cn1[:], cmean[:], psRA[0:D, 0:NB], A.mult)
                cn16 = sm1.tile([D, NB], BF16, tag="cn16")
                nc.vector.tensor_tensor(cn16[:], cn1[:], psRB[0:D, 0:NB],
                                        A.subtract)
                psH = psp.tile([128, CW], F32, tag="ps")
                nc.tensor.matmul(psH[0:NCLS, 0:NB], wt("headW"), cn16[:],
                                 start=True, stop=True)
                hout = sm1.tile([NCLS, NB], F32, tag="hout")
                nc.scalar.activation(hout[:], psH[0:NCLS, 0:NB], AF.Identity,
                                     bias=wt("head_bias"), scale=1.0)
                nc.sync.dma_start(yout[:], hout[:])
    nc.compile()
    return nc, dbg_t


# ---------------- PJRT runner (inlined) ----------------
import time as _time
import jax as _jax


def make_runner(nc, n_cores):
    from concourse import bass2jax, mybir
    bass2jax.install_neuronx_cc_hook()
    assert nc.dbg_addr is None or not nc.dbg_callbacks
    partition_name = nc.partition_id_tensor.name if nc.partition_id_tensor else None

    in_names, out_names, out_avals, zero_outs = [], [], [], []
    for alloc in nc.m.functions[0].allocations:
        if not isinstance(alloc, mybir.MemoryLocationSet):
            continue
        name = alloc.memorylocations[0].name
        if alloc.kind == "ExternalInput":
            if name != partition_name and name != (
                nc.dbg_addr.name if nc.dbg_addr else None
            ):
                in_names.append(name)
        elif alloc.kind == "ExternalOutput":
            out_names.append(name)
            np_dt = mybir.dt.np(alloc.dtype)
            out_avals.append(
                _jax.core.ShapedArray(tuple(alloc.tensor_shape), np_dt)
            )
            zero_outs.append(np.zeros(tuple(alloc.tensor_shape), np_dt))

    n_params = len(in_names)
    all_in_names = list(in_names) + list(out_names)
    if nc.dbg_addr is not None:
        all_in_names.append(nc.dbg_addr.name)
    if partition_name is not None:
        all_in_names.append(partition_name)

    def _body(*args):
        operands = list(args)
        if nc.dbg_addr is not None:
            operands.append(np.zeros((1, 2), np.uint32))
        if partition_name is not None:
            operands.append(bass2jax.partition_id_tensor())
        outs = bass2jax._bass_exec_p.bind(
            *operands,
            out_avals=tuple(out_avals),
            in_names=tuple(all_in_names),
            out_names=tuple(out_names),
            lowering_input_output_aliases=(),
            sim_require_finite=True,
            sim_require_nnan=True,
            nc=nc,
        )
        return tuple(outs)

    if n_cores == 1:
        jitted = _jax.jit(_body, keep_unused=True)

        def run(in_map):
            args = [np.asarray(in_map[n]) for n in in_names] + zero_outs
            outs = jitted(*args)
            return {n: np.asarray(o) for n, o in zip(out_names, outs)}
    else:
        from jax.sharding import Mesh, PartitionSpec
        from jax.experimental.shard_map import shard_map

        devices = _jax.devices()[:n_cores]
        mesh = Mesh(np.asarray(devices), ("core",))
        n_outs = len(out_names)
        in_specs = (PartitionSpec("core"),) * (n_params + n_outs)
        out_specs = (PartitionSpec("core"),) * n_outs
        jitted = _jax.jit(
            shard_map(_body, mesh=mesh, in_specs=in_specs, out_specs=out_specs,
                      check_rep=False),
            keep_unused=True,
        )

        from jax.sharding import NamedSharding
        shard = NamedSharding(mesh, PartitionSpec("core"))
        dev_cache = {}

        def run(in_maps):
            concat_in = []
            for n in in_names:
                if n == "x":
                    concat_in.append(np.concatenate(
                        [np.asarray(m[n]) for m in in_maps], axis=0))
                else:
                    a = dev_cache.get(n)
                    if a is None:
                        a = _jax.device_put(np.concatenate(
                            [np.asarray(m[n]) for m in in_maps], axis=0), shard)
                        dev_cache[n] = a
                    concat_in.append(a)
            concat_zero = [
                np.zeros((n_cores * z.shape[0], *z.shape[1:]), z.dtype)
                for z in zero_outs
            ]
            outs = jitted(*concat_in, *concat_zero)
            out_np = [np.asarray(o) for o in outs]
            result = []
            for c in range(n_cores):
                result.append({
                    n: o[c * z.shape[0]:(c + 1) * z.shape[0]]
                    for n, o, z in zip(out_names, out_np, zero_outs)
                })
            return result

    return run


_CACHE = {}


def kernel(**inputs):
    if "runner" not in _CACHE:
        nc, _ = build()
        _CACHE["runner"] = make_runner(nc, NCORES)
    run = _CACHE["runner"]
    p = prep_weights(inputs)
    x = np.asarray(inputs["x"], np.float32).astype(ml_dtypes.bfloat16)
    in_maps = []
    for ci in range(NCORES):
        m = dict(p)
        m["x"] = x[ci * NB:(ci + 1) * NB]
        in_maps.append(m)
    res = run(in_maps)
    return np.concatenate([r["y"].T for r in res], 0).astype(np.float32)

